# revision 2
# baseline (speedup 1.0000x reference)
"""Dice-coefficient-mean kernel for Trainium2 (8 NeuronCores, SPMD), v3.

Sharding: data-parallel over batch — core b processes batch b
([128, 16384] layout per tensor).

40 exact integer statistics per core:
  inter[l] = #(pair == 17l), l=0..13       (pair = 16*s1 + s2, bf16 exact)
  F1[T] = #(s1 <= T), F2[T] = #(s2 <= T), T = 0..12
F1[12]/F2[12] recovered from free linear stats sum(s1)/sum(s2)
(PE ones-matmul on raw data): sum(x) = 13N - sum_{T<=12} F[T].

Engine allocation (HW-measured: DVE bf16 mask 4.3us, PE 32-MM ones-chain
6.9us, ACT Sign+accum 14.2us per full stat):
  - ScalarE: F2[0..K_ACT-1] via Sign(x-(T+.5)) free-dim accumulators,
    plus the phase-1 PSUM drains (activation Copy + accum from PSUM).
  - DVE->PE: F1 + inter as bf16 {0,1} masks, each reduced by a single
    PSUM [1,512] matmul chain (one drain per stat). N_FOLD inter stats
    are folded 2:1 on DVE (TT add of halves -> {0,1,2}) to halve PE
    cost; folds are interleaved among full stats to keep PE fed.
  - PE-only: sum(s1)/sum(s2) chains on raw data during the DMA phase.
  - Input DMA on the sync (HWDGE) ring: lower first-byte latency; one
    ring already saturates the 358 GB/s per-core HBM read cap.
Counts stay exact (PSUM slot <= 13*4096 < 2^24); host math in float64.
"""

import numpy as np

NUM_LABELS = 14
EPS = float(np.finfo(float).eps)
B = 8
P = 128
FREE = 16384
NCHUNK = 4
CF = FREE // NCHUNK  # 4096
MM_N = 512
L = NUM_LABELS

K_ACT = 13    # stats on ScalarE (F2 cumulative first, then F1)
N_FOLD = 6    # inter stats on the DVE fold path
GROUP_A = 6   # F1 stats computed chunk-wise during the DMA phase

_CACHE = {}


def _plan(k_act=K_ACT, n_fold=N_FOLD, group_a=GROUP_A):
    """Returns (phase1, phase2, act_thresholds).

    mask spec: (kind, value, folded), kind in {f1, f2, inter}.
    psum col order = [sum1, sum2] + phase1 + phase2 (in emission order).
    """
    n_act_f2 = min(k_act, L - 2)
    n_act_f1 = k_act - n_act_f2
    act = [("s2", t) for t in range(n_act_f2)]
    act += [("s1", L - 3 - i) for i in range(n_act_f1)]
    act_f1 = {t for (s, t) in act if s == "s1"}
    f1 = [("f1", float(t), False) for t in range(L - 2) if t not in act_f1]
    f2 = [("f2", float(t), False) for t in range(n_act_f2, L - 2)]
    inter_full = [("inter", 17.0 * l, False) for l in range(L - n_fold)]
    inter_fold = [("inter", 17.0 * l, True) for l in range(L - n_fold, L)]
    phase1 = f1[:group_a]
    fulls = f1[group_a:] + f2 + inter_full
    # interleave folds among fulls, none in the last TAIL_FULL slots
    TAIL_FULL = 3
    head_fulls = fulls[: max(0, len(fulls) - TAIL_FULL)]
    phase2 = []
    nf, nd = len(head_fulls), len(inter_fold)
    fi = di = 0
    for k in range(nf + nd):
        if (di * nf <= fi * nd and di < nd and fi > 0) or fi >= nf:
            phase2.append(inter_fold[di])
            di += 1
        else:
            phase2.append(head_fulls[fi])
            fi += 1
    phase2 += fulls[len(head_fulls):]
    return phase1, phase2, act


def _build(k_act=K_ACT, n_fold=N_FOLD, group_a=GROUP_A):
    from concourse import bacc, mybir, tile

    op = mybir.AluOpType
    af = mybir.ActivationFunctionType
    dt = mybir.dt

    phase1, phase2, act_thr = _plan(k_act, n_fold, group_a)
    nps = 2 + len(phase1) + len(phase2)

    nc = bacc.Bacc("TRN2", target_bir_lowering=False)
    s1 = nc.dram_tensor("s1", [P, FREE], dt.float32, kind="ExternalInput")
    s2 = nc.dram_tensor("s2", [P, FREE], dt.float32, kind="ExternalInput")
    out_p = nc.dram_tensor("stats_pe", [1, nps], dt.float32, kind="ExternalOutput")
    out_a = nc.dram_tensor(
        "stats_act", [P, max(2 * k_act, 1)], dt.float32, kind="ExternalOutput"
    )

    HALF = FREE // 2

    def sl(c):
        return slice(c * CF, (c + 1) * CF)

    with tile.TileContext(nc) as tc:
        with (
            tc.tile_pool(name="aux", bufs=1) as aux,
            tc.tile_pool(name="mask", bufs=5) as maskp,
            tc.tile_pool(name="cmask", bufs=2) as cmaskp,
            tc.tile_pool(name="psum", bufs=8, space="PSUM") as psum,
        ):
            s1h = aux.tile([P, FREE], dt.float16)
            s2h = aux.tile([P, FREE], dt.float16)
            pair = aux.tile([P, FREE], dt.float16)
            junk = aux.tile([P, 3 * CF], dt.float8e4)
            drjunk = aux.tile([1, MM_N], dt.float32)
            ones = aux.tile([P, 1], dt.float16)
            stats_p = aux.tile([1, nps], dt.float32)
            stats_a = aux.tile([P, max(2 * k_act, 1)], dt.float32)
            nc.vector.memset(ones[:], 1.0)
            biases = aux.tile([P, max(k_act, 1)], dt.float32)
            for i, (s, t) in enumerate(act_thr):
                nc.vector.memset(biases[:, i : i + 1], -(t + 0.5))

            # s1*16 staging shares the mask pool (slots recycled for masks)
            s1_16a = maskp.tile([P, HALF], dt.float16, tag="mask")
            s1_16b = maskp.tile([P, HALF], dt.float16, tag="mask")

            accs = {}

            def chain(name, srct, width, c0, cN):
                acc = accs[name]
                nmm = width // MM_N
                for k in range(nmm):
                    nc.tensor.matmul(
                        acc[:],
                        ones[:],
                        srct[:, k * MM_N : (k + 1) * MM_N],
                        start=(c0 and k == 0),
                        stop=(cN and k == nmm - 1),
                        skip_group_check=True,
                    )

            def drain_v(name, col):
                nc.vector.tensor_reduce(
                    out=stats_p[:1, col : col + 1],
                    in_=accs[name][:1, :],
                    axis=mybir.AxisListType.X,
                    op=op.add,
                )

            def drain_s(name, col):
                nc.scalar.activation(
                    out=drjunk[:],
                    in_=accs[name][:1, :],
                    func=af.Copy,
                    bias=0.0,
                    scale=1.0,
                    accum_out=stats_p[:1, col : col + 1],
                )

            # ---- phase 1: streamed over chunks ----
            accs["sum1"] = psum.tile([1, MM_N], dt.float32, tag="acc", name="a_s1")
            accs["sum2"] = psum.tile([1, MM_N], dt.float32, tag="acc", name="a_s2")
            for i, _ in enumerate(phase1):
                accs[f"p1_{i}"] = psum.tile(
                    [1, MM_N], dt.float32, tag="acc", name=f"a_p1{i}"
                )

            for c in range(NCHUNK):
                if c == 0:
                    nc.gpsimd.dma_start(out=s2h[:, sl(c)], in_=s2[:, sl(c)])
                    nc.gpsimd.dma_start(out=s1h[:, sl(c)], in_=s1[:, sl(c)])
                else:
                    nc.gpsimd.dma_start(out=s1h[:, sl(c)], in_=s1[:, sl(c)])
                    nc.gpsimd.dma_start(out=s2h[:, sl(c)], in_=s2[:, sl(c)])
                if c == 0:
                    for i, (s, t) in enumerate(act_thr):
                        nc.scalar.activation(
                            out=junk[:, :CF],
                            in_=(s2h if s == "s2" else s1h)[:, sl(0)],
                            func=af.Sign, bias=biases[:, i : i + 1],
                            scale=1.0,
                            accum_out=stats_a[:, 2 * i : 2 * i + 1],
                        )
                s1_16 = s1_16a if c < 2 else s1_16b
                csl = slice((c % 2) * CF, (c % 2) * CF + CF)
                nc.vector.tensor_scalar(
                    out=s1_16[:, csl], in0=s1h[:, sl(c)],
                    scalar1=16.0, scalar2=None, op0=op.mult,
                )
                nc.vector.tensor_add(
                    out=pair[:, sl(c)], in0=s1_16[:, csl], in1=s2h[:, sl(c)]
                )
                chain("sum1", s1h[:, sl(c)], CF, c == 0, c == NCHUNK - 1)
                chain("sum2", s2h[:, sl(c)], CF, c == 0, c == NCHUNK - 1)
                for i, (kind, thr, _f) in enumerate(phase1):
                    m = cmaskp.tile([P, CF], dt.float16, tag="cm")
                    nc.vector.tensor_scalar(
                        out=m[:], in0=s1h[:, sl(c)],
                        scalar1=thr, scalar2=None, op0=op.is_le,
                    )
                    chain(f"p1_{i}", m, CF, c == 0, c == NCHUNK - 1)

            # ScalarE: the wide ACT stats (no drains here — avoids bubble)
            cols = {}
            p1_names = ["sum1", "sum2"] + [f"p1_{i}" for i in range(len(phase1))]
            for i, name in enumerate(p1_names):
                cols[name] = i
            next_col = 2 + len(phase1)
            for i, (s, t) in enumerate(act_thr):
                nc.scalar.activation(
                    out=junk[:, : 3 * CF],
                    in_=(s2h if s == "s2" else s1h)[:, CF:FREE],
                    func=af.Sign, bias=biases[:, i : i + 1], scale=1.0,
                    accum_out=stats_a[:, 2 * i + 1 : 2 * i + 2],
                )

            # ---- phase 2: full-width masks, folds interleaved ----
            drain_q = list(p1_names)
            emit_idx = {}
            for j, (kind, val, folded) in enumerate(phase2):
                name = f"p2_{j}"
                cols[name] = next_col
                next_col += 1
                accs[name] = psum.tile(
                    [1, MM_N], dt.float32, tag="acc", name=f"a_{name}"
                )
                src = {"f1": s1h, "f2": s2h, "inter": pair}[kind]
                mop = op.is_le if kind in ("f1", "f2") else op.is_equal
                ma = maskp.tile([P, HALF], dt.float16, tag="mask")
                mb = maskp.tile([P, HALF], dt.float16, tag="mask")
                nc.vector.tensor_scalar(
                    out=ma[:], in0=src[:, :HALF],
                    scalar1=val, scalar2=None, op0=mop,
                )
                nc.vector.tensor_scalar(
                    out=mb[:], in0=src[:, HALF:],
                    scalar1=val, scalar2=None, op0=mop,
                )
                if folded:
                    fm = maskp.tile([P, HALF], dt.float16, tag="mask")
                    nc.vector.tensor_tensor(
                        out=fm[:], in0=ma[:], in1=mb[:], op=op.add,
                    )
                    chain(name, fm, HALF, True, True)
                else:
                    chain(name, ma, HALF, True, False)
                    chain(name, mb, HALF, False, True)
                emit_idx[name] = j
                drain_q.append(name)
                budget = 2
                while drain_q and budget > 0:
                    d = drain_q[0]
                    if d not in p1_names and j - emit_idx[d] < 2:
                        break
                    drain_q.pop(0)
                    drain_v(d, cols[d])
                    budget -= 1
            for d in drain_q:
                drain_v(d, cols[d])

            nc.sync.dma_start(out=out_p[:], in_=stats_p[:])
            nc.sync.dma_start(out=out_a[:], in_=stats_a[:])
    nc.compile()
    return nc


def _get_built(k_act=K_ACT, n_fold=N_FOLD, group_a=GROUP_A):
    key = (k_act, n_fold, group_a)
    if key not in _CACHE:
        _CACHE[key] = _build(*key)
    return _CACHE[key]


LAST_EXEC_NS = None
LAST_RESULTS = None


def _decode(results, k_act=K_ACT, n_fold=N_FOLD, group_a=GROUP_A):
    phase1, phase2, act_thr = _plan(k_act, n_fold, group_a)
    n = float(P * FREE)
    dice = np.zeros((B, NUM_LABELS), dtype=np.float64)
    for b in range(B):
        sp = np.asarray(results[b]["stats_pe"], dtype=np.float64).ravel()
        sa = np.asarray(results[b]["stats_act"], dtype=np.float64)
        sum1, sum2 = sp[0], sp[1]
        f1 = np.zeros(NUM_LABELS)
        f2 = np.zeros(NUM_LABELS)
        inter = np.zeros(NUM_LABELS)
        specs = list(phase1) + list(phase2)
        for (kind, val, _f), v in zip(specs, sp[2 : 2 + len(specs)]):
            if kind == "f1":
                f1[int(round(val))] = v
            elif kind == "f2":
                f2[int(round(val))] = v
            else:
                inter[int(round(val / 17.0))] = v
        for i, (srcname, t) in enumerate(act_thr):
            s = sa[:, 2 * i : 2 * i + 2].sum()  # sum of signs = #gt - #le
            if srcname == "s2":
                f2[t] = (n - s) / 2.0
            else:
                f1[t] = (n - s) / 2.0
        f1[L - 2] = 13.0 * n - sum1 - f1[: L - 2].sum()
        f2[L - 2] = 13.0 * n - sum2 - f2[: L - 2].sum()
        f1[L - 1] = n
        f2[L - 1] = n
        c1 = np.diff(f1, prepend=0.0)
        c2 = np.diff(f2, prepend=0.0)
        dice[b] = 2.0 * inter / (c1 + c2 + EPS)
    resv = dice.reshape(-1)
    total = resv.sum()
    nz = float((resv > 0).sum())
    mean = total / nz if nz > 0 else 0.0
    return np.float32(mean)


def _run(segment1, segment2, trace=False):
    global LAST_EXEC_NS, LAST_RESULTS
    from concourse.bass_utils import run_bass_kernel_spmd

    nc = _get_built()
    seg1 = np.ascontiguousarray(np.asarray(segment1, dtype=np.float32)).reshape(
        B, P, FREE
    )
    seg2 = np.ascontiguousarray(np.asarray(segment2, dtype=np.float32)).reshape(
        B, P, FREE
    )
    in_maps = [{"s1": seg1[b], "s2": seg2[b]} for b in range(B)]
    res = run_bass_kernel_spmd(nc, in_maps, core_ids=list(range(B)), trace=trace)
    LAST_EXEC_NS = res.exec_time_ns
    LAST_RESULTS = res
    return _decode(res.results)


def kernel(segment1, segment2):
    return _run(segment1, segment2, trace=False)


def benchmark(segment1, segment2):
    try:
        _run(segment1, segment2, trace=True)
    except Exception:
        _run(segment1, segment2, trace=False)
    return LAST_EXEC_NS
